# revision 1
# baseline (speedup 1.0000x reference)
"""AdaptiveAttention (B=2, S=2048, D=1024, H=16) on 8 TRN2 NeuronCores.

Sharding: query-parallel. Core c (c = 0..7) owns batch b = c//4 and query rows
[512*(c%4), 512*(c%4+1)). K/V projections are computed for the core's own 512
key rows and AllGathered (bf16) within each batch group of 4 cores, split into
8 per-head-pair pieces so the collectives pipeline behind compute. Each core then
computes all 16 heads of attention for its 512 queries against the full 2048
keys, applies the per-(head, query) sigmoid-gate / softmax-denominator scale
to the context, and runs the full output projection for its rows. The host
concatenates the 8 disjoint [512, 1024] output blocks.

On-chip layout is "feature-major" (transposed): projections produce Q^T/K^T
directly so scores come out keys-on-partitions, which feeds exp (ScalarE, with
the 1/sqrt(dk) folded into the activation scale) and the P@V matmul without
any on-device transposes. Matmuls are bf16 with f32 PSUM accumulation. Scores
are column-packed into array quadrants; P@V column-packs the two heads of a
pair; softmax denominators are ones-vector matmuls column-packed 4-wide per
pair (even/odd key tiles split across column groups, recombined on VectorE).
"""

import contextlib
import ctypes
import os
import sys
import types

import numpy as np
import ml_dtypes


def _install_ntff_hook_shim():
    """Provide antenv.axon_hooks (absent in this image) so
    run_bass_kernel_spmd(trace=True) can capture NTFF profiles."""
    if "antenv.axon_hooks" in sys.modules:
        return
    try:
        lib = ctypes.CDLL("/opt/axon/libaxon_pjrt.so")
    except OSError:
        return
    if not hasattr(lib, "axon_start_nrt_profile"):
        return
    lib.axon_start_nrt_profile.argtypes = [
        ctypes.POINTER(ctypes.c_int64),
        ctypes.c_size_t,
    ]
    lib.axon_start_nrt_profile.restype = ctypes.c_int64
    lib.axon_stop_nrt_profile.argtypes = [ctypes.c_char_p]
    lib.axon_stop_nrt_profile.restype = ctypes.c_int64

    @contextlib.contextmanager
    def _hook(output_dir, device_ids):
        import jax

        jax.devices()
        if device_ids:
            ids = (ctypes.c_int64 * len(device_ids))(*device_ids)
            rc = lib.axon_start_nrt_profile(ids, len(device_ids))
        else:
            rc = lib.axon_start_nrt_profile(None, 0)
        if rc != 0:
            raise RuntimeError(f"axon_start_nrt_profile rc={rc}")
        try:
            yield
        finally:
            n = lib.axon_stop_nrt_profile(str(output_dir).encode())
            if n < 0:
                raise RuntimeError(f"axon_stop_nrt_profile rc={n}")

    mod = types.ModuleType("antenv.axon_hooks")
    _state = {"hook": _hook}
    mod.get_axon_ntff_profile_hook = lambda: _state["hook"]
    mod.set_axon_ntff_profile_hook = lambda h: _state.__setitem__("hook", h)
    sys.modules["antenv.axon_hooks"] = mod
    try:
        import antenv

        antenv.axon_hooks = mod
    except ImportError:
        pass


_install_ntff_hook_shim()

import concourse.bass as bass  # noqa: E402
import concourse.mybir as mybir  # noqa: E402
import concourse.tile as tile  # noqa: E402
from concourse import bacc  # noqa: E402
from concourse.bass_utils import run_bass_kernel_spmd  # noqa: E402

# ---------------------------------------------------------------------------
# Problem constants (hardcoded; kernel.py must be self-contained)
# ---------------------------------------------------------------------------
B, S, D, H = 2, 2048, 1024, 16
DK = D // H                  # 64
N_CORES = 8
R = 4                        # ranks per batch group
SL = S // R                  # 512 local rows per core
P = 128
DT = D // P                  # 8 feature tiles
NKT = S // P                 # 16 key tiles
SCALE = DK ** -0.5

F32 = mybir.dt.float32
BF16 = mybir.dt.bfloat16
AF = mybir.ActivationFunctionType
BF16_NP = ml_dtypes.bfloat16

_CACHE = {}
LAST_EXEC_TIME_NS = None


def _build():
    nc = bacc.Bacc("TRN2", target_bir_lowering=False, debug=False,
                   num_devices=N_CORES)

    # ---- I/O --------------------------------------------------------------
    xqT = nc.dram_tensor("xqT", [D, SL], BF16, kind="ExternalInput")
    xkT = nc.dram_tensor("xkT", [D, SL], BF16, kind="ExternalInput")
    xvT = nc.dram_tensor("xvT", [D, SL], BF16, kind="ExternalInput")
    wq = nc.dram_tensor("wq", [D, D], BF16, kind="ExternalInput")
    wk = nc.dram_tensor("wk", [D, D], BF16, kind="ExternalInput")
    wv = nc.dram_tensor("wv", [D, D], BF16, kind="ExternalInput")
    wo = nc.dram_tensor("wo", [D, D], BF16, kind="ExternalInput")
    wg = nc.dram_tensor("wg", [P, DT, H], BF16, kind="ExternalInput")
    bq = nc.dram_tensor("bq", [P, DT], F32, kind="ExternalInput")
    bk = nc.dram_tensor("bk", [P, DT], F32, kind="ExternalInput")
    bvb = nc.dram_tensor("bvb", [P, D], BF16, kind="ExternalInput")
    bob = nc.dram_tensor("bob", [P, D], BF16, kind="ExternalInput")
    bg = nc.dram_tensor("bg", [H, 1], F32, kind="ExternalInput")
    out = nc.dram_tensor("out", [SL, D], F32, kind="ExternalOutput")

    with tile.TileContext(nc) as tc:
        with (
            tc.tile_pool(name="cst", bufs=1) as cst,
            tc.tile_pool(name="wpool", bufs=2) as wpool,
            tc.tile_pool(name="xpool", bufs=2) as xpool,
            tc.tile_pool(name="kvpool", bufs=4) as kvpool,
            tc.tile_pool(name="work", bufs=2) as work,
            tc.tile_pool(name="work2", bufs=1) as work2,
            tc.tile_pool(name="pt_pool", bufs=4) as ptp,
            tc.tile_pool(name="psA", bufs=1, space="PSUM") as psA,
            tc.tile_pool(name="psB", bufs=3, space="PSUM") as psB,
            tc.tile_pool(name="psC", bufs=1, space="PSUM") as psC,
            tc.tile_pool(name="dram", bufs=1, space="DRAM") as dram,
        ):
            # psB "sc" tiles are [P, 2, 512]; projection chains borrow one
            # 512-column slice of the same slots so wave-phase scores get
            # 3-deep pipelining against exp within the 8-bank PSUM budget.
            def proj_psum():
                t = psB.tile([P, 2, 512], F32, tag="sc")
                return t[:, 0, :]

            # ---- chunked input loads (first matmuls start early) ----------
            def load_w(dram_t):  # rotating weight slot [128, 8, 1024]
                t = wpool.tile([P, DT, D], BF16, tag="wmat")
                src3 = dram_t.ap().rearrange("(t p) f -> p t f", p=P)
                for c in range(4):
                    nc.sync.dma_start(t[:, 2 * c:2 * c + 2, :],
                                      src3[:, 2 * c:2 * c + 2, :])
                return t

            def load_x(dram_t):  # rotating activation slot [128, 8, 512]
                t = xpool.tile([P, DT, SL], BF16, tag="xmat")
                src3 = dram_t.ap().rearrange("(t p) f -> p t f", p=P)
                for c in range(4):
                    nc.scalar.dma_start(t[:, 2 * c:2 * c + 2, :],
                                        src3[:, 2 * c:2 * c + 2, :])
                return t

            bk_sb = cst.tile([P, DT], F32, name="bk_sb")
            nc.sync.dma_start(bk_sb[:], bk[:])
            bv_sb = cst.tile([P, D], BF16, name="bv_sb")
            nc.sync.dma_start(bv_sb[:], bvb[:])
            wg_sb = cst.tile([P, DT, H], BF16, name="wg_sb")
            nc.sync.dma_start(wg_sb[:], wg[:])
            bq_sb = cst.tile([P, DT], F32, name="bq_sb")
            nc.sync.dma_start(bq_sb[:], bq[:])
            bg_sb = cst.tile([H, 1], F32, name="bg_sb")
            nc.sync.dma_start(bg_sb[:], bg[:])
            ones_sb = cst.tile([P, 1], BF16, name="ones_sb")
            nc.vector.memset(ones_sb[:], 1.0)
            wk_sb = wpool.tile([P, DT, D], BF16, tag="wmat", name="wk_sb")
            wv_sb = wpool.tile([P, DT, D], BF16, tag="wmat", name="wv_sb")
            xk_sb = xpool.tile([P, DT, SL], BF16, tag="xmat", name="xk_sb")
            xv_sb = xpool.tile([P, DT, SL], BF16, tag="xmat", name="xv_sb")
            wq_sb = cst.tile([P, DT, D], BF16, name="wq_cst")
            xq_sb = cst.tile([P, DT, SL], BF16, name="xq_cst")
            def _src(d):
                return d.ap().rearrange("(t p) f -> p t f", p=P)

            # interleave so K-proj inputs land first, then V, then Q
            for c in range(4):
                sl = slice(2 * c, 2 * c + 2)
                nc.sync.dma_start(wk_sb[:, sl, :], _src(wk)[:, sl, :])
                nc.scalar.dma_start(xk_sb[:, sl, :], _src(xkT)[:, sl, :])
                nc.sync.dma_start(wv_sb[:, sl, :], _src(wv)[:, sl, :])
                nc.scalar.dma_start(xv_sb[:, sl, :], _src(xvT)[:, sl, :])
            for c in range(4):
                sl = slice(2 * c, 2 * c + 2)
                nc.sync.dma_start(wq_sb[:, sl, :], _src(wq)[:, sl, :])
                nc.scalar.dma_start(xq_sb[:, sl, :], _src(xqT)[:, sl, :])

            # ---- K^T / V projections + per-wave pipelined AllGather ------
            # piece w carries K^T dims-tiles {2w, 2w+1} and V dim-window
            # [256w, 256w+256) for the core's 512 local keys (512KB/rank).
            ktloc = kvpool.tile([P, DT, SL], BF16, tag="kv")
            # window-major V: [p, window(=dims-tile), kb, 128] so per-piece
            # bounce DMAs read contiguous spans
            vloc = kvpool.tile([P, DT, R, P], BF16, tag="kv")
            # AG pieces: (dims-tile start, end). Small leading pieces let the
            # first head-pairs start while later pieces are still in flight.
            PIECES = [(i, i + 1) for i in range(8)]
            PIECE_OF_PAIR = list(range(8))
            in_b = [dram.tile([2, P, (pe - ps) * SL], BF16, name=f"in_b{i}")
                    for i, (ps, pe) in enumerate(PIECES)]
            out_b = [dram.tile([R, 2, P, (pe - ps) * SL], BF16,
                               name=f"out_b{i}")
                     for i, (ps, pe) in enumerate(PIECES)]

            def k_proj(mt):
                pp = proj_psum()
                for kt in range(DT):
                    nc.tensor.matmul(pp[:], wk_sb[:, kt, 128 * mt:128 * mt + 128],
                                     xk_sb[:, kt, :],
                                     start=(kt == 0), stop=(kt == DT - 1))
                nc.vector.tensor_scalar_add(ktloc[:, mt, :], pp[:],
                                            bk_sb[:, mt:mt + 1])

            def v_proj(kb, c2):
                pp = proj_psum()
                for kt in range(DT):
                    nc.tensor.matmul(
                        pp[:], xv_sb[:, kt, 128 * kb:128 * kb + 128],
                        wv_sb[:, kt, 512 * c2:512 * c2 + 512],
                        start=(kt == 0), stop=(kt == DT - 1))
                nc.vector.tensor_add(
                    vloc[:, 4 * c2:4 * c2 + 4, kb, :],
                    pp[:].rearrange("p (w d) -> p w d", w=4),
                    bv_sb[:, 512 * c2:512 * c2 + 512].rearrange(
                        "p (w d) -> p w d", w=4))

            def issue_piece(i):
                ps_, pe_ = PIECES[i]
                n = pe_ - ps_
                nc.gpsimd.dma_start(
                    in_b[i][0].rearrange("p (t k) -> p t k", t=n),
                    ktloc[:, ps_:pe_, :])
                nc.gpsimd.dma_start(
                    in_b[i][1].rearrange("p (w a d) -> p w a d", w=n, a=R),
                    vloc[:, ps_:pe_, :, :])
                nc.gpsimd.collective_compute(
                    "AllGather",
                    mybir.AluOpType.bypass,
                    replica_groups=[[0, 1, 2, 3], [4, 5, 6, 7]],
                    ins=[in_b[i].opt()],
                    outs=[out_b[i].opt()],
                )

            k_proj(0)
            for kb in range(R):
                v_proj(kb, 0)
            issue_piece(0)
            for mt in (1, 2, 3):
                k_proj(mt)
                issue_piece(mt)
            k_proj(4)
            for kb in range(R):
                v_proj(kb, 1)
            issue_piece(4)
            for mt in (5, 6, 7):
                k_proj(mt)
                issue_piece(mt)
            # ---- Q^T projection + gate (overlap the in-flight AllGathers)

            qt_sb = cst.tile([P, DT, SL], BF16, name="qt_sb")
            for mt in range(DT):
                pp = proj_psum()
                for kt in range(DT):
                    nc.tensor.matmul(pp[:], wq_sb[:, kt, 128 * mt:128 * mt + 128],
                                     xq_sb[:, kt, :],
                                     start=(kt == 0), stop=(kt == DT - 1))
                nc.vector.tensor_scalar_add(qt_sb[:, mt, :], pp[:],
                                            bq_sb[:, mt:mt + 1])

            gate_sb = cst.tile([H, SL], F32, name="gate_sb")
            gp = psC.tile([H, 512], F32, tag="sums")
            for kt in range(DT):
                nc.tensor.matmul(gp[:], wg_sb[:, kt, :], xq_sb[:, kt, :],
                                 start=(kt == 0), stop=(kt == DT - 1))
            nc.scalar.activation(gate_sb[:], gp[:], AF.Sigmoid,
                                 bias=bg_sb[:, 0:1])


            # ---- attention, 8 head-pairs pipelined -----------------------

            ctxT = cst.tile([P, DT, SL], BF16, name="ctxT")

            def emit_pv(st, tg):
                # P@V + denominator matmuls for 2 keytiles of a prior pair
                for par in (0, 1):
                    tau = 2 * tg + par
                    vt = st["vw"][:, tau // R, st["lp"], tau % R, :]
                    nc.tensor.matmul(
                        st["cp"][64:128, :], vt[:, 64:128],
                        st["ptB"][:, tau, :],
                        start=(tau == 0), stop=(tau == NKT - 1),
                        tile_position=(0, 64))
                    nc.tensor.matmul(
                        st["cp"][0:64, :], vt[:, 0:64],
                        st["ptA"][:, tau, :],
                        start=(tau == 0), stop=(tau == NKT - 1),
                        tile_position=(0, 0))
                for j, pt_t in ((0, st["ptA"]), (1, st["ptB"])):
                    for par in (0, 1):
                        colg = 2 * j + par
                        tau = 2 * tg + par
                        nc.tensor.matmul(
                            st["sums"][32 * colg:32 * colg + 1, :],
                            ones_sb[:, 0:1], pt_t[:, tau, :],
                            start=(tg == 0), stop=(tg == NKT // 2 - 1),
                            tile_position=(0, 32 * colg))

            def emit_scale(st):
                pair, hA, hB = st["pair"], st["hA"], st["hB"]
                sums_pp = st["sums"]
                # free the ctx PSUM bank first; scale in-place below
                nc.vector.tensor_copy(ctxT[:, pair, :], st["cp"][:, :])
                rec = work2.tile([P, 512], F32, tag="rec")
                nc.vector.tensor_copy(rec[0:32, :], sums_pp[32:64, :])
                nc.vector.tensor_copy(rec[64:96, :], sums_pp[96:128, :])
                nc.vector.tensor_add(rec[0:96, :], sums_pp[0:96, :],
                                     rec[0:96, :])
                nc.vector.reciprocal(rec[0:1, :], rec[0:1, :])
                nc.vector.reciprocal(rec[64:65, :], rec[64:65, :])
                gal = work2.tile([P, 512], F32, tag="gal")
                nc.sync.dma_start(gal[0:1, :], gate_sb[hA:hA + 1, :])
                nc.sync.dma_start(gal[64:65, :], gate_sb[hB:hB + 1, :])
                nc.vector.tensor_mul(gal[0:1, :], gal[0:1, :], rec[0:1, :])
                nc.vector.tensor_mul(gal[64:65, :], gal[64:65, :],
                                     rec[64:65, :])
                sbc = work.tile([P, 512], F32, tag="sbc")
                srow_d = dram.tile([2, 512], F32, name=f"srow_d{pair}")
                nc.sync.dma_start(srow_d[0:1, :], gal[0:1, :])
                nc.sync.dma_start(srow_d[1:2, :], gal[64:65, :])
                nc.sync.dma_start(sbc[0:64, :],
                                  srow_d[0:1, :].to_broadcast([64, 512]))
                nc.sync.dma_start(sbc[64:128, :],
                                  srow_d[1:2, :].to_broadcast([64, 512]))
                nc.vector.tensor_mul(ctxT[:, pair, :],
                                     ctxT[:, pair, :], sbc[:, :])

            cur = [-1, None, None, 0]  # piece idx, ktw, vw, piece dt-start
            pend = None
            for pair in range(DT):
                pc = PIECE_OF_PAIR[pair]
                if pc != cur[0]:
                    ps_, pe_ = PIECES[pc]
                    n = pe_ - ps_
                    ktw = kvpool.tile([P, n, R, SL], BF16, tag="kv",
                                      name=f"ktw{pc}")
                    vw = kvpool.tile([P, R, n, R, P], BF16, tag="kv",
                                     name=f"vw{pc}")
                    for r_ in range(R):
                        nc.gpsimd.dma_start(
                            ktw[:, :, r_, :],
                            out_b[pc][r_, 0].rearrange("p (t k) -> p t k", t=n))
                        nc.gpsimd.dma_start(
                            vw[:, r_, :, :, :],
                            out_b[pc][r_, 1].rearrange(
                                "p (w a d) -> p w a d", w=n, a=R))
                    cur = [pc, ktw, vw, ps_]
                ktw, vw, ps_ = cur[1], cur[2], cur[3]
                lp = pair - ps_
                hA, hB = 2 * pair, 2 * pair + 1
                ptA = ptp.tile([P, NKT, SL], BF16, tag="pt")
                ptB = ptp.tile([P, NKT, SL], BF16, tag="pt")
                st = {
                    "pair": pair, "hA": hA, "hB": hB, "lp": lp,
                    "vw": vw, "ptA": ptA, "ptB": ptB,
                    "cp": psA.tile([P, 512], F32, tag="pc", name="cp_ps"),
                    "sums": psC.tile([P, 512], F32, tag="sums", name="sums_ps"),
                }
                is_last = pair == DT - 1
                for tg in range(NKT // 2):
                    sA = psB.tile([P, 2, 512], F32, tag="sc")
                    sB = psB.tile([P, 2, 512], F32, tag="sc")
                    for j in (0, 1):
                        tau = 2 * tg + j
                        r_, kl = tau // R, tau % R
                        klo = slice(128 * kl, 128 * kl + 64)
                        khi = slice(128 * kl + 64, 128 * kl + 128)
                        nc.tensor.matmul(
                            sB[0:64, j, :], ktw[64:128, lp, r_, klo],
                            qt_sb[64:128, pair, :],
                            start=True, stop=True, tile_position=(64, 0))
                        nc.tensor.matmul(
                            sB[64:128, j, :], ktw[64:128, lp, r_, khi],
                            qt_sb[64:128, pair, :],
                            start=True, stop=True, tile_position=(64, 64))
                        nc.tensor.matmul(
                            sA[0:64, j, :], ktw[0:64, lp, r_, klo],
                            qt_sb[0:64, pair, :],
                            start=True, stop=True, tile_position=(0, 0))
                        nc.tensor.matmul(
                            sA[64:128, j, :], ktw[0:64, lp, r_, khi],
                            qt_sb[0:64, pair, :],
                            start=True, stop=True, tile_position=(0, 64))
                    nc.scalar.activation(ptA[:, 2 * tg:2 * tg + 2, :],
                                         sA[:, :, :], AF.Exp, scale=SCALE)
                    nc.scalar.activation(ptB[:, 2 * tg:2 * tg + 2, :],
                                         sB[:, :, :], AF.Exp, scale=SCALE)
                    if pend is not None:
                        emit_pv(pend, tg)
                    if is_last:
                        # final pair consumes its own tiles right away
                        emit_pv(st, tg)
                if pend is not None:
                    emit_scale(pend)
                pend = None if is_last else st
            emit_scale(st)

            # ---- output projection --------------------------------------
            wo_sb = load_w(wo)
            bo_sb = cst.tile([P, D], BF16, name="bo_sb")
            nc.sync.dma_start(bo_sb[:], bob[:])
            for qi in range(SL // P):
                for c2 in range(2):
                    po = proj_psum()
                    for pair in range(DT):
                        nc.tensor.matmul(
                            po[:], ctxT[:, pair, 128 * qi:128 * qi + 128],
                            wo_sb[:, pair, 512 * c2:512 * c2 + 512],
                            start=(pair == 0), stop=(pair == DT - 1))
                    osb = work.tile([P, 512], F32, tag="osb")
                    nc.vector.tensor_add(osb[:, :],
                                         po[:], bo_sb[:, 512 * c2:512 * c2 + 512])
                    nc.sync.dma_start(
                        out[128 * qi:128 * qi + 128, 512 * c2:512 * c2 + 512],
                        osb[:])

    nc.compile()
    return nc


def _prep_inputs(query, key_, value, Wq, bq, Wk, bk, Wv, bv, Wo, bo, Wg, bg):
    """Host-side sharding / layout prep. Returns in_maps for the 8 cores."""
    f32 = np.float32

    def bf(x):
        return np.ascontiguousarray(np.asarray(x, f32)).astype(BF16_NP)

    wq_b, wk_b, wv_b, wo_b = bf(Wq), bf(Wk), bf(Wv), bf(Wo)
    wg_b = np.ascontiguousarray(bf(Wg).reshape(DT, P, H).transpose(1, 0, 2))
    bq_pm = np.ascontiguousarray(np.asarray(bq, f32).reshape(DT, P).T)
    bk_pm = np.ascontiguousarray(np.asarray(bk, f32).reshape(DT, P).T)
    bv_b = np.ascontiguousarray(
        np.broadcast_to(np.asarray(bv, f32).astype(BF16_NP), (P, D)))
    bo_b = np.ascontiguousarray(
        np.broadcast_to(np.asarray(bo, f32).astype(BF16_NP), (P, D)))
    bg_c = np.ascontiguousarray(np.asarray(bg, f32).reshape(H, 1))

    qT = [np.asarray(query[b], f32).T for b in range(B)]
    kT = [np.asarray(key_[b], f32).T for b in range(B)]
    vT = [np.asarray(value[b], f32).T for b in range(B)]

    in_maps = []
    for c in range(N_CORES):
        b, r = c // R, c % R
        rows = slice(SL * r, SL * (r + 1))
        in_maps.append({
            "xqT": np.ascontiguousarray(qT[b][:, rows]).astype(BF16_NP),
            "xkT": np.ascontiguousarray(kT[b][:, rows]).astype(BF16_NP),
            "xvT": np.ascontiguousarray(vT[b][:, rows]).astype(BF16_NP),
            "wq": wq_b, "wk": wk_b, "wv": wv_b, "wo": wo_b, "wg": wg_b,
            "bq": bq_pm, "bk": bk_pm, "bvb": bv_b, "bob": bo_b, "bg": bg_c,
        })
    return in_maps


def kernel(query, key_, value, Wq, bq, Wk, bk, Wv, bv, Wo, bo, Wg, bg):
    global LAST_EXEC_TIME_NS
    if "nc" not in _CACHE:
        _CACHE["nc"] = _build()
    nc = _CACHE["nc"]

    in_maps = _prep_inputs(query, key_, value, Wq, bq, Wk, bk, Wv, bv,
                           Wo, bo, Wg, bg)
    trace = bool(os.environ.get("BASS_TRACE"))
    res = run_bass_kernel_spmd(nc, in_maps, core_ids=list(range(N_CORES)),
                               trace=trace)
    LAST_EXEC_TIME_NS = res.exec_time_ns

    out = np.empty((B, S, D), np.float32)
    for c in range(N_CORES):
        b, r = c // R, c % R
        out[b, SL * r:SL * (r + 1), :] = res.results[c]["out"]
    return out

